# revision 4
# baseline (speedup 1.0000x reference)
"""GNN message-passing aggregator on 8 Trainium2 NeuronCores.

Reference computation (single device):
    deg     = bincount(edge_src)                      # out-degree, >= 1
    s       = 1/sqrt(deg)
    msg_e   = entity_embed[src_e] * s[src_e]
    agg_v   = sum_{e: dst_e == v} msg_e
    out_v   = s[v] * agg_v

Device strategy (dst-sharded, edge-parallel):
  * Nodes are grouped into 128-row blocks. Blocks are dealt to the 8 cores
    (sorted by edge count, one block per core per "position") so that every
    core runs the identical compiled program: position i has CPB[i] chunks
    of 128 edges on every core.
  * Per 128-edge chunk: the 128 source rows are gathered from the (replicated)
    embedding table in DRAM with one batched indirect DMA; a fused DVE
    tensor_scalar builds a scaled one-hot [128 edges, 128 node-offsets]
    ((iota == dstoff) * s_src); the tensor engine matmul-accumulates
    one_hot^T @ rows into the block's PSUM mailbox [128 nodes, 64].
  * Block eviction applies the per-node dst scale (per-partition scalar) and
    the whole per-core output [128, NPOS*64] is written back with one DMA.
  * Host does index-only prep (degree counts, sort/shard/pad) and the final
    unshard (block permutation back to node order).
"""
import sys

sys.path.insert(0, "/opt/trn_rl_repo")

import numpy as np

N_NODES = 100_000
N_EDGES = 1_000_000
D = 64
P = 128
NCORES = 8
K = 32  # chunks per batched gather (128*K rows per indirect DMA)


def _prep(entity_embed, edge_src, edge_dst):
    """Host-side index prep + sharding. Returns (in_maps, meta)."""
    n_blk_real = -(-N_NODES // P)  # 782
    n_blk = -(-n_blk_real // NCORES) * NCORES  # 784
    npos = n_blk // NCORES  # 98

    deg = np.bincount(edge_src, minlength=N_NODES).astype(np.float64)
    inv_sqrt = (1.0 / np.sqrt(deg)).astype(np.float32)  # deg >= 1 guaranteed

    blk = (edge_dst // P).astype(np.int64)
    cnt = np.bincount(blk, minlength=n_blk)  # edges per block
    cpb = -(-cnt // P)  # chunks per block (0 possible)

    # order edges grouped by block
    order = np.argsort(blk, kind="stable")
    starts = np.zeros(n_blk + 1, np.int64)
    starts[1:] = np.cumsum(cnt)

    # deal blocks: sort desc by chunk count, groups of 8 -> one per core
    bsort = np.argsort(-cpb, kind="stable")
    block_of = bsort.reshape(npos, NCORES)  # [pos, core] -> global block
    CPB = cpb[block_of[:, 0]]  # chunks at each position (max of its group)
    CPB = np.maximum(CPB, 1)  # keep >=1 so every position has a matmul group
    S = np.zeros(npos + 1, np.int64)
    S[1:] = np.cumsum(CPB)
    C = int(S[-1])
    C_pad = -(-C // K) * K

    tab = entity_embed
    if tab.shape[0] != n_blk * P:
        tab = np.zeros((n_blk * P, D), entity_embed.dtype)
        tab[:N_NODES] = entity_embed

    in_maps = []
    for c in range(NCORES):
        src_a = np.zeros((P, C_pad), np.int32)
        doff_a = np.zeros((P, C_pad), np.float32)
        ssrc_a = np.zeros((P, C_pad), np.float32)
        sdst_a = np.zeros((P, npos), np.float32)
        for i in range(npos):
            b = block_of[i, c]
            e = order[starts[b]:starts[b + 1]]
            t = np.arange(e.size)
            rows = t % P
            cols = S[i] + t // P
            es = edge_src[e]
            src_a[rows, cols] = es
            doff_a[rows, cols] = (edge_dst[e] - b * P).astype(np.float32)
            ssrc_a[rows, cols] = inv_sqrt[es]
            nodes = b * P + np.arange(P)
            valid = nodes < N_NODES
            sdst_a[valid, i] = inv_sqrt[nodes[valid]]
        in_maps.append(
            {
                "table": tab,
                "src": src_a,
                "dstoff": doff_a,
                "ssrc": ssrc_a,
                "sdst": sdst_a,
            }
        )
    meta = dict(npos=npos, CPB=CPB, C_pad=C_pad, block_of=block_of, n_blk=n_blk)
    return in_maps, meta


def _build(meta):
    import concourse.bass as bass
    import concourse.bacc as bacc
    import concourse.mybir as mybir
    import concourse.tile as tile

    npos = meta["npos"]
    CPB = meta["CPB"]
    C_pad = meta["C_pad"]
    n_blk = meta["n_blk"]
    f32 = mybir.dt.float32

    nc = bacc.Bacc("TRN2", target_bir_lowering=False, debug=False)
    t_tab = nc.dram_tensor("table", [n_blk * P, D], f32, kind="ExternalInput")
    t_src = nc.dram_tensor("src", [P, C_pad], mybir.dt.int32, kind="ExternalInput")
    t_doff = nc.dram_tensor("dstoff", [P, C_pad], f32, kind="ExternalInput")
    t_ssrc = nc.dram_tensor("ssrc", [P, C_pad], f32, kind="ExternalInput")
    t_sdst = nc.dram_tensor("sdst", [P, npos], f32, kind="ExternalInput")
    t_out = nc.dram_tensor("out", [P, npos * D], f32, kind="ExternalOutput")

    with tile.TileContext(nc) as tc:
        with (
            tc.tile_pool(name="const", bufs=1) as cpool,
            tc.tile_pool(name="g", bufs=8) as gpool,
            tc.tile_pool(name="oh", bufs=6) as ohpool,
            tc.tile_pool(name="psum", bufs=4, space="PSUM") as ppool,
            tc.tile_pool(name="outp", bufs=1) as opool,
        ):
            idx_sb = cpool.tile([P, C_pad], mybir.dt.int32)
            doff_sb = cpool.tile([P, C_pad], f32)
            ssrc_sb = cpool.tile([P, C_pad], f32)
            sdst_sb = cpool.tile([P, npos], f32)
            iota_i = cpool.tile([P, P], mybir.dt.int32)
            iota_f = cpool.tile([P, P], f32)
            out_sb = opool.tile([P, npos * D], f32)

            nc.sync.dma_start(out=idx_sb[:], in_=t_src[:])
            nc.sync.dma_start(out=doff_sb[:], in_=t_doff[:])
            nc.sync.dma_start(out=ssrc_sb[:], in_=t_ssrc[:])
            nc.sync.dma_start(out=sdst_sb[:], in_=t_sdst[:])
            nc.gpsimd.iota(iota_i[:], pattern=[[1, P]], base=0, channel_multiplier=0)
            nc.vector.tensor_copy(out=iota_f[:], in_=iota_i[:])

            j = 0
            for i in range(npos):
                psum = ppool.tile([P, D], f32)
                for k in range(int(CPB[i])):
                    g = gpool.tile([P, D], f32, tag="g")
                    nc.gpsimd.indirect_dma_start(
                        out=g[:],
                        out_offset=None,
                        in_=t_tab[:],
                        in_offset=bass.IndirectOffsetOnAxis(
                            ap=idx_sb[:, j:j + 1], axis=0
                        ),
                    )
                    oh = ohpool.tile([P, P], f32, tag="oh")
                    nc.vector.tensor_scalar(
                        out=oh[:],
                        in0=iota_f[:],
                        scalar1=doff_sb[:, j:j + 1],
                        scalar2=ssrc_sb[:, j:j + 1],
                        op0=mybir.AluOpType.is_equal,
                        op1=mybir.AluOpType.mult,
                    )
                    nc.tensor.matmul(
                        out=psum[:],
                        lhsT=oh[:],
                        rhs=g[:],
                        start=(k == 0),
                        stop=(k == int(CPB[i]) - 1),
                    )
                    j += 1
                nc.vector.tensor_scalar(
                    out=out_sb[:, i * D:(i + 1) * D],
                    in0=psum[:],
                    scalar1=sdst_sb[:, i:i + 1],
                    scalar2=None,
                    op0=mybir.AluOpType.mult,
                )
            nc.sync.dma_start(out=t_out[:], in_=out_sb[:])
    nc.finalize()
    return nc


def _unshard(results, meta):
    npos = meta["npos"]
    block_of = meta["block_of"]
    n_blk = meta["n_blk"]
    full = np.zeros((n_blk * P, D), np.float32)
    for c in range(NCORES):
        o = np.asarray(results[c]["out"]).reshape(P, npos, D).transpose(1, 0, 2)
        full[(block_of[:, c][:, None] * P + np.arange(P)[None, :]).ravel()] = (
            o.reshape(npos * P, D)
        )
    return full[:N_NODES]


def _run(entity_embed, edge_src, edge_dst, trace=False):
    from concourse import bass_utils

    in_maps, meta = _prep(
        np.asarray(entity_embed, np.float32),
        np.asarray(edge_src),
        np.asarray(edge_dst),
    )
    nc = _build(meta)
    res = bass_utils.run_bass_kernel_spmd(
        nc, in_maps, list(range(NCORES)), trace=trace
    )
    return _unshard(res.results, meta), res


def kernel(entity_embed, edge_src, edge_dst):
    out, _ = _run(entity_embed, edge_src, edge_dst)
    return out
